# revision 12
# baseline (speedup 1.0000x reference)
"""Trainium2 Bass kernel for nn_CustomTransformerLayer (rel-pos transformer layer).

Sharding: 8 cores = batch(4) x query-half(2). Each core computes 128 query rows
of one batch element end-to-end (attention w/ rel-pos terms + FF, post-LN).

Core-local algorithm (q on partitions, d_rel layout: head h -> cols 32h..32h+32):
  qkv_q   = x_q @ Win.T + b      (token-major, for q2 / QhT)
  qkvT    = strip-wise Win.T @ xfull (d-major, K/V strips only, quadrant-aligned)
  scores  = QhT_h.T @ KT_h  (PE, K=32)  + S_rel (DVE: rq*q2 product + seg-reduce)
  softmax = max / exp(scale*(s-m)) with accum_out sum / recip   (ACT+DVE)
  aT      = PE transposes of A; val_main = aT.T @ V_tok         (PE)
  val_rel = per-q PE matmul aT_q.T @ rv_q -> [8h',256d] psum groups of 4 q
            -> blockmask mult + gsel group-reduce (diag select)
  out     = LN2(x2 + FF(LN1(x_q + val @ Wout.T)))
"""

import os
import sys

import numpy as np

sys.path.insert(0, "/opt/trn_rl_repo")

B, T, D, H, HD, F = 4, 256, 256, 8, 32, 1024
SCALE = 1.0 / float(np.sqrt(HD))
EPS = 1e-5
NCORES = 8
KB = 8          # rq k-rows per DMA/product block
F32 = None      # set lazily (mybir.dt.float32)

_CACHE = {}
LAST_RESULTS = None


def _build_program():
    import concourse.bass as bass
    import concourse.bacc as bacc
    import concourse.mybir as mybir
    from concourse.masks import make_identity
    from concourse.tile import TileContext

    f32 = mybir.dt.float32
    AX = mybir.AxisListType
    AF = mybir.ActivationFunctionType

    nc = bacc.Bacc()

    # ---- DRAM params -------------------------------------------------------
    x_q = nc.declare_dram_parameter("x_q", [128, D], f32, isOutput=False)
    xfull = nc.declare_dram_parameter("xfull", [T, D], f32, isOutput=False)
    rq = nc.declare_dram_parameter("rq", [128, T, D], f32, isOutput=False)
    rv = nc.declare_dram_parameter("rv", [128, T, D], f32, isOutput=False)
    w_in = nc.declare_dram_parameter("in_proj_w", [3 * D, D], f32, isOutput=False)
    b_in = nc.declare_dram_parameter("in_proj_b", [3 * D], f32, isOutput=False)
    w_out = nc.declare_dram_parameter("out_proj_w", [D, D], f32, isOutput=False)
    b_out = nc.declare_dram_parameter("out_proj_b", [D], f32, isOutput=False)
    w1 = nc.declare_dram_parameter("lin1_w", [F, D], f32, isOutput=False)
    b1 = nc.declare_dram_parameter("lin1_b", [F], f32, isOutput=False)
    w2 = nc.declare_dram_parameter("lin2_w", [D, F], f32, isOutput=False)
    b2 = nc.declare_dram_parameter("lin2_b", [D], f32, isOutput=False)
    ln1_g = nc.declare_dram_parameter("ln1_g", [D], f32, isOutput=False)
    ln1_b = nc.declare_dram_parameter("ln1_b", [D], f32, isOutput=False)
    ln2_g = nc.declare_dram_parameter("ln2_g", [D], f32, isOutput=False)
    ln2_b = nc.declare_dram_parameter("ln2_b", [D], f32, isOutput=False)
    bmask = nc.declare_dram_parameter("bmask", [128, D], f32, isOutput=False)
    gsel = nc.declare_dram_parameter("gsel", [128, 4], f32, isOutput=False)
    out = nc.declare_dram_parameter("out", [128, D], f32, isOutput=True)

    with TileContext(nc) as tc:
        with (
            tc.tile_pool(name="const", bufs=1) as cpool,
            tc.tile_pool(name="wstage", bufs=2) as wstage,
            tc.tile_pool(name="rqp", bufs=3) as rqpool,
            tc.tile_pool(name="rvp", bufs=8) as rvpool,
            tc.tile_pool(name="prod", bufs=2) as prodpool,
            tc.tile_pool(name="work", bufs=2) as work,
            tc.tile_pool(name="small", bufs=4) as small,
            tc.tile_pool(name="ps", bufs=2, space="PSUM") as ps,
            tc.tile_pool(name="psv", bufs=1, space="PSUM") as psv,
            tc.tile_pool(name="ps4", bufs=2, space="PSUM") as ps4,
        ):
            _ctr = [0]

            def _nm(p):
                _ctr[0] += 1
                return f"{p}_{_ctr[0]}"

            def t256():
                return ps.tile([128, 256], f32, tag="t256", name=_nm("t256"))

            def t128():
                return ps.tile([128, 128], f32, tag="t128", name=_nm("t128"))

            def copy(dst, src):
                nc.scalar.copy(out=dst, in_=src)

            # ---- constants & weights --------------------------------------
            ident = cpool.tile([128, 128], f32)
            make_identity(nc, ident)
            ones = cpool.tile([1, 256], f32)
            nc.vector.memset(ones, 1.0)
            eps_sb = cpool.tile([128, 1], f32)
            nc.vector.memset(eps_sb, EPS)
            bmask_sb = cpool.tile([128, D], f32)
            nc.sync.dma_start(bmask_sb, bmask[:, :])
            gsel_sb = cpool.tile([128, 4], f32)
            nc.sync.dma_start(gsel_sb, gsel[:, :])

            bin_sb = cpool.tile([1, 3 * D], f32)
            nc.sync.dma_start(bin_sb, b_in[None, :])
            bout_sb = cpool.tile([1, D], f32)
            nc.sync.dma_start(bout_sb, b_out[None, :])
            b1_sb = cpool.tile([1, F], f32)
            nc.sync.dma_start(b1_sb, b1[None, :])
            b2_sb = cpool.tile([1, D], f32)
            nc.sync.dma_start(b2_sb, b2[None, :])

            # LN gamma/beta broadcast to [128, D] via ones-matmul
            ln_bc = {}
            for name, param in (("g1", ln1_g), ("b1", ln1_b),
                                ("g2", ln2_g), ("b2", ln2_b)):
                row = cpool.tile([1, D], f32, tag=f"lnrow_{name}")
                nc.sync.dma_start(row, param[None, :])
                pt = t256()
                nc.tensor.matmul(pt, lhsT=ones[:, :128], rhs=row, start=True, stop=True)
                bc = cpool.tile([128, D], f32, tag=f"lnbc_{name}")
                copy(bc, pt)
                ln_bc[name] = bc

            def transpose_to(dst_sb, src_sb):
                """[128, n<=128] sbuf -> psum transpose -> dst sbuf [n, 128]."""
                pt = t128()
                n = src_sb.shape[-1]
                nc.tensor.transpose(pt[:n, :], src_sb, ident)
                copy(dst_sb, pt[:n, :])

            # in_proj_w [768, 256] -> inprojT [128, 2, 768]  ([d_in, dh, d_out])
            inprojT = cpool.tile([128, 2, 3 * D], f32)
            for m in range(6):
                nat = wstage.tile([128, D], f32, tag="nat")
                nc.sync.dma_start(nat, w_in[128 * m:128 * (m + 1), :])
                for dh in range(2):
                    transpose_to(inprojT[:, dh, 128 * m:128 * (m + 1)],
                                 nat[:, 128 * dh:128 * (dh + 1)])

            # lin1_w [1024, 256] -> W1T [128, 2, 1024]
            W1T = cpool.tile([128, 2, F], f32)
            for m in range(8):
                nat = wstage.tile([128, D], f32, tag="nat")
                nc.sync.dma_start(nat, w1[128 * m:128 * (m + 1), :])
                for dh in range(2):
                    transpose_to(W1T[:, dh, 128 * m:128 * (m + 1)],
                                 nat[:, 128 * dh:128 * (dh + 1)])

            # lin2_w [256, 1024] -> W2T [128, 8, 256]
            W2T = cpool.tile([128, 8, D], f32)
            for m in range(2):
                nat2 = wstage.tile([128, F], f32, tag="nat2")
                nc.sync.dma_start(nat2, w2[128 * m:128 * (m + 1), :])
                for fh in range(8):
                    transpose_to(W2T[:, fh, 128 * m:128 * (m + 1)],
                                 nat2[:, 128 * fh:128 * (fh + 1)])

            # out_proj_w [256, 256] -> OWT [128, 2, 256]
            OWT = cpool.tile([128, 2, D], f32)
            for m in range(2):
                nat = wstage.tile([128, D], f32, tag="nat")
                nc.sync.dma_start(nat, w_out[128 * m:128 * (m + 1), :])
                for dh in range(2):
                    transpose_to(OWT[:, dh, 128 * m:128 * (m + 1)],
                                 nat[:, 128 * dh:128 * (dh + 1)])

            # ---- x transposes ---------------------------------------------
            # xfull -> xT [128, 2, 256]  ([d_in, db, tok])
            xT = cpool.tile([128, 2, T], f32)
            for tt in range(2):
                natx = wstage.tile([128, D], f32, tag="natx")
                nc.sync.dma_start(natx, xfull[128 * tt:128 * (tt + 1), :])
                for db in range(2):
                    transpose_to(xT[:, db, 128 * tt:128 * (tt + 1)],
                                 natx[:, 128 * db:128 * (db + 1)])

            xq_sb = cpool.tile([128, D], f32)
            nc.sync.dma_start(xq_sb, x_q[:, :])
            xqT = cpool.tile([128, 2, 128], f32)
            for db in range(2):
                transpose_to(xqT[:, db, :], xq_sb[:, 128 * db:128 * (db + 1)])

            # ---- QKV ------------------------------------------------------
            # K/V strips (d-major, quadrant-aligned into head-group psums)
            # strip rows R..R+32 of in_proj output; R = 96h + 32*typ (typ: 1=K, 2=V)
            KT_rel = cpool.tile([128, 2, T], f32)   # [32(h%4)+j, h//4, tok]
            VT_rel = cpool.tile([128, 2, T], f32)
            for typ, dest in ((1, KT_rel), (2, VT_rel)):
                for hg in range(2):                  # head group h = 4*hg + i
                    pt = t256()
                    for i in range(4):
                        h = 4 * hg + i
                        R = 96 * h + 32 * typ
                        sl = pt[32 * i:32 * i + 32, :]
                        for dh in range(2):
                            nc.tensor.matmul(
                                sl,
                                lhsT=inprojT[:, dh, R:R + 32],
                                rhs=xT[:, dh, :],
                                start=(dh == 0), stop=False,
                                tile_position=(0, 32 * i))
                        nc.tensor.matmul(
                            sl, lhsT=bin_sb[:, R:R + 32], rhs=ones,
                            start=False, stop=True,
                            tile_position=(0, 32 * i))
                    copy(dest[:, hg, :], pt)

            # qkv_q token-major [128 tok, 768] (for q2 / QhT)
            qkvq_sb = cpool.tile([128, 3 * D], f32)
            for nb in range(3):
                pt = t256()
                for dh in range(2):
                    nc.tensor.matmul(
                        pt, lhsT=xqT[:, dh, :],
                        rhs=inprojT[:, dh, 256 * nb:256 * (nb + 1)],
                        start=(dh == 0), stop=False)
                nc.tensor.matmul(
                    pt, lhsT=ones[:, :128],
                    rhs=bin_sb[:, 256 * nb:256 * (nb + 1)],
                    start=False, stop=True)
                copy(qkvq_sb[:, 256 * nb:256 * (nb + 1)], pt)

            # q2 [128 tok, 256 d_rel]: head h <- qkvq cols 96h..96h+32
            q2 = cpool.tile([128, D], f32)
            qv = qkvq_sb[:].rearrange("p (h r) -> p h r", h=8)[:, :, 0:32]
            copy(q2[:].rearrange("p (h j) -> p h j", h=8), qv)

            # QhT [32(h%4)+j, h//4, 128 q]  (d-major q strips for scores lhsT)
            QhT = cpool.tile([128, 2, 128], f32)
            for hg in range(2):
                pt = t128()
                for i in range(4):
                    h = 4 * hg + i
                    # normal matmul vs identity == transpose, but allows
                    # nonzero PSUM partition offset (verifier restriction)
                    nc.tensor.matmul(
                        pt[32 * i:32 * i + 32, :],
                        lhsT=q2[:, 32 * h:32 * h + 32], rhs=ident,
                        start=True, stop=True,
                        tile_position=(0, 32 * i))
                copy(QhT[:, hg, :], pt)

            # V_tok [128 tok%128, kb, 256 d_rel]
            V_tok = cpool.tile([128, 2, D], f32)
            for kb in range(2):
                for dh in range(2):
                    transpose_to(V_tok[:, kb, 128 * dh:128 * (dh + 1)],
                                 VT_rel[:, dh, 128 * kb:128 * (kb + 1)])

            # ---- rel_q einsum + scores + softmax --------------------------
            S_rel = cpool.tile([128, T, 8], f32)     # [q, k, h]
            for kb in range(T // KB):
                rq_t = rqpool.tile([128, KB, D], f32)
                nc.sync.dma_start(rq_t, rq[:, KB * kb:KB * (kb + 1), :])
                prod = prodpool.tile([128, KB, D], f32)
                nc.vector.tensor_mul(
                    prod, rq_t,
                    q2[:, None, :].to_broadcast([128, KB, D]))
                nc.vector.reduce_sum(
                    S_rel[:, KB * kb:KB * (kb + 1), :],
                    prod[:].rearrange("p k (h j) -> p k h j", j=HD),
                    axis=AX.X)

            A = cpool.tile([128, 8, T], f32)         # exp(scale*(s-m)), unnormalized
            rinv = cpool.tile([128, 8], f32)         # 1/sum per (q, h)
            for h in range(8):
                pt = t256()
                off = 32 * (h % 4)
                nc.tensor.matmul(
                    pt,
                    lhsT=QhT[off:off + 32, h // 4, :],
                    rhs=KT_rel[off:off + 32, h // 4, :],
                    start=True, stop=True,
                    tile_position=(off, 0))
                sc = work.tile([128, T], f32, tag="sc")
                nc.vector.tensor_add(sc, pt, S_rel[:, :, h])
                mx = small.tile([128, 1], f32, tag="mx")
                nc.vector.reduce_max(mx, sc, axis=AX.X)
                negms = small.tile([128, 1], f32, tag="negms")
                nc.vector.tensor_scalar_mul(negms, mx, -SCALE)
                ssum = small.tile([128, 1], f32, tag="ssum")
                nc.scalar.activation(
                    out=A[:, h, :], in_=sc, func=AF.Exp,
                    bias=negms, scale=SCALE, accum_out=ssum)
                nc.vector.reciprocal(rinv[:, h:h + 1], ssum)

            # ---- aT + val_main --------------------------------------------
            aT = cpool.tile([128, 2, 8, 128], f32)   # [k%128, kb, h, q]
            for kb in range(2):
                for h in range(8):
                    pt = t128()
                    nc.tensor.transpose(
                        pt, A[:, h, 128 * kb:128 * (kb + 1)], ident)
                    copy(aT[:, kb, h, :], pt)

            vmain = psv.tile([128, D], f32)
            for h in range(8):
                for kb in range(2):
                    nc.tensor.matmul(
                        vmain[:, 32 * h:32 * h + 32],
                        lhsT=aT[:, kb, h, :],
                        rhs=V_tok[:, kb, 32 * h:32 * h + 32],
                        start=(kb == 0), stop=(kb == 1))

            # ---- val_rel: per-q matmuls, groups of 4 q --------------------
            vrel_full = cpool.tile([128, D], f32)
            vrel_stage = cpool.tile([4, 32, D], f32)
            for t in range(32):
                pg = ps.tile([128, 256], f32, tag="t256", name=_nm("pg"))
                nc.vector.memset(pg, 0.0)
                for g in range(4):
                    q = 4 * t + g
                    rv_t = rvpool.tile([128, 2, D], f32)
                    nc.sync.dma_start(
                        rv_t, rv[q].rearrange("(kb p) d -> p kb d", p=128))
                    for kb in range(2):
                        nc.tensor.matmul(
                            pg[32 * g:32 * g + 8, :],
                            lhsT=aT[:, kb, :, q],
                            rhs=rv_t[:, kb, :],
                            start=(kb == 0), stop=(kb == 1),
                            skip_group_check=True,
                            tile_position=(0, 32 * g))
                masked = work.tile([128, D], f32, tag="masked")
                nc.vector.tensor_mul(masked, pg, bmask_sb)
                p4 = ps4.tile([4, D], f32, tag="p4", name=_nm("p4"))
                nc.tensor.matmul(p4, lhsT=gsel_sb, rhs=masked,
                                 start=True, stop=True)
                copy(vrel_stage[:, t, :], p4)

            # reassemble vrel_stage [p, t, d] -> vrel_full rows q = 4t+p
            for t in range(32):
                nc.sync.dma_start(vrel_full[4 * t:4 * t + 4, :],
                                  vrel_stage[:, t, :])

            # ---- val = (vmain + vrel) * rinv ------------------------------
            val = cpool.tile([128, D], f32)
            nc.vector.tensor_add(val, vmain, vrel_full)
            v3 = val[:].rearrange("p (h j) -> p h j", h=8)
            nc.vector.tensor_mul(
                v3, v3, rinv[:, :, None].to_broadcast([128, 8, HD]))

            def layer_norm(dst, src, g_bc, b_bc):
                stats = small.tile([128, 6], f32, tag="bnst")
                nc.vector.bn_stats(out=stats, in_=src)
                mv = small.tile([128, 2], f32, tag="bnagg")
                nc.vector.bn_aggr(out=mv, in_=stats)
                rstd = small.tile([128, 1], f32, tag="rstd")
                nc.scalar.activation(
                    out=rstd, in_=mv[:, 1:2], func=AF.Sqrt,
                    bias=eps_sb[:, :], scale=1.0)
                nc.vector.reciprocal(rstd, rstd)
                cen = work.tile([128, D], f32, tag="cen")
                nc.vector.tensor_scalar(
                    out=cen, in0=src, scalar1=mv[:, 0:1], scalar2=rstd,
                    op0=mybir.AluOpType.subtract, op1=mybir.AluOpType.mult)
                nc.vector.tensor_mul(cen, cen, g_bc)
                nc.vector.tensor_add(dst, cen, b_bc)

            # ---- out_proj + residual + LN1 --------------------------------
            valT = cpool.tile([128, 2, 128], f32)
            for dh in range(2):
                transpose_to(valT[:, dh, :], val[:, 128 * dh:128 * (dh + 1)])
            pt_att = t256()
            for dh in range(2):
                nc.tensor.matmul(pt_att, lhsT=valT[:, dh, :], rhs=OWT[:, dh, :],
                                 start=(dh == 0), stop=False)
            nc.tensor.matmul(pt_att, lhsT=ones[:, :128], rhs=bout_sb,
                             start=False, stop=True)
            h1 = cpool.tile([128, D], f32)
            nc.vector.tensor_add(h1, pt_att, xq_sb)
            x1 = cpool.tile([128, D], f32)
            layer_norm(x1, h1, ln_bc["g1"], ln_bc["b1"])

            # ---- FF -------------------------------------------------------
            x1T = cpool.tile([128, 2, 128], f32)
            for dh in range(2):
                transpose_to(x1T[:, dh, :], x1[:, 128 * dh:128 * (dh + 1)])
            hsb = cpool.tile([128, F], f32)
            for fq in range(4):
                pt = t256()
                for dh in range(2):
                    nc.tensor.matmul(
                        pt, lhsT=x1T[:, dh, :],
                        rhs=W1T[:, dh, 256 * fq:256 * (fq + 1)],
                        start=(dh == 0), stop=False)
                nc.tensor.matmul(
                    pt, lhsT=ones[:, :128],
                    rhs=b1_sb[:, 256 * fq:256 * (fq + 1)],
                    start=False, stop=True)
                nc.scalar.activation(
                    out=hsb[:, 256 * fq:256 * (fq + 1)], in_=pt, func=AF.Relu)

            hT = cpool.tile([128, 8, 128], f32)
            for fh in range(8):
                transpose_to(hT[:, fh, :], hsb[:, 128 * fh:128 * (fh + 1)])
            pt_ff = t256()
            for fh in range(8):
                nc.tensor.matmul(pt_ff, lhsT=hT[:, fh, :], rhs=W2T[:, fh, :],
                                 start=(fh == 0), stop=False)
            nc.tensor.matmul(pt_ff, lhsT=ones[:, :128], rhs=b2_sb,
                             start=False, stop=True)
            h2 = cpool.tile([128, D], f32)
            nc.vector.tensor_add(h2, pt_ff, x1)
            out_sb = cpool.tile([128, D], f32)
            layer_norm(out_sb, h2, ln_bc["g2"], ln_bc["b2"])
            nc.sync.dma_start(out[:, :], out_sb)

    nc.finalize()
    return nc


def _host_consts():
    p = np.arange(128)
    d = np.arange(D)
    bmask = ((p[:, None] % 32) == (d[None, :] // HD)).astype(np.float32)
    gsel = ((p[:, None] // 32) == np.arange(4)[None, :]).astype(np.float32)
    return bmask, gsel


def kernel(**inputs):
    global LAST_RESULTS
    from concourse.bass_utils import run_bass_kernel_spmd

    if "nc" not in _CACHE:
        _CACHE["nc"] = _build_program()
    nc = _CACHE["nc"]

    inp = {k: np.ascontiguousarray(np.asarray(v, dtype=np.float32))
           for k, v in inputs.items()}
    bmask, gsel = _host_consts()

    weight_names = ["in_proj_w", "in_proj_b", "out_proj_w", "out_proj_b",
                    "lin1_w", "lin1_b", "lin2_w", "lin2_b",
                    "ln1_g", "ln1_b", "ln2_g", "ln2_b"]
    in_maps = []
    for c in range(NCORES):
        b, qh = c // 2, c % 2
        m = {
            "x_q": inp["query"][b, 128 * qh:128 * (qh + 1)],
            "xfull": inp["query"][b],
            "rq": inp["rel_pos_q"][b, 128 * qh:128 * (qh + 1)],
            "rv": inp["rel_pos_v"][b, 128 * qh:128 * (qh + 1)],
            "bmask": bmask,
            "gsel": gsel,
        }
        for wn in weight_names:
            m[wn] = inp[wn]
        in_maps.append({k: np.ascontiguousarray(v) for k, v in m.items()})

    trace = os.environ.get("BASS_KERNEL_TRACE") == "1"
    res = run_bass_kernel_spmd(nc, in_maps, list(range(NCORES)), trace=trace)
    LAST_RESULTS = res

    out = np.zeros((B, T, D), np.float32)
    for c in range(NCORES):
        b, qh = c // 2, c % 2
        out[b, 128 * qh:128 * (qh + 1)] = res.results[c]["out"]
    return out
